# revision 1
# baseline (speedup 1.0000x reference)
"""Trainium2 Bass kernel for nn_Attention_Critic (8-agent attention critic).

Data-parallel over batch across 8 NeuronCores (2048 batch rows per core).
BatchNorm statistics are made global via a tiny AllReduce (8x160 f32).

Device layout: feature-on-partitions, [feat, batch] tiles.
  - host pre-transposes s|a to [agent, 80, 2048] per core (bf16)
  - encoders / projections: PE matmuls, contraction on partitions
  - attention einsum pibd,pjbd->pijb: DVE pairwise muls on [128=(head,d), B]
    tiles + PE block-ones matmuls reducing over d (partition reduce) into a
    compact [32=(4j+p), B] logits block
  - softmax: ACT exp, PE ones-matmul for the j-sum, DVE reciprocal; weights
    broadcast d-wise back to [128, B] by a second ones-pattern matmul
  - weighted sum over agents folded into the critic's PSUM accumulation
  - argmax-gather of q: exact first-max logic with DVE compare/reduce ops
"""

import sys

sys.path.insert(0, "/opt/trn_rl_repo")

import numpy as np
from ml_dtypes import bfloat16

from concourse import bacc, bass, mybir, tile
from concourse.bass_utils import run_bass_kernel_spmd

F32 = mybir.dt.float32
BF16 = mybir.dt.bfloat16
AF = mybir.ActivationFunctionType
ALU = mybir.AluOpType
AX = mybir.AxisListType

N_AGENTS = 8
BATCH = 16384
SDIM, ADIM = 64, 16
CDIM = SDIM + ADIM  # 80
HID = 128
HEADS = 4
ATT_D = 32
N_CORES = 8
SH = BATCH // N_CORES  # 2048 batch rows per core
NH = 2  # batch halves per core
BH = SH // NH  # 1024
NC512 = BH // 512  # matmul chunks per half
EPS = 1e-5
ISQD = float(1.0 / np.sqrt(np.float32(ATT_D)))
SLOPE = 0.01  # LeakyReLU negative slope

_CACHE = {}
DBG = False


def _build():
    nc = bacc.Bacc(None, num_devices=N_CORES)

    # ---- DRAM parameters (per-core shard shapes) ----
    xt = nc.declare_dram_parameter("xt", [N_AGENTS, CDIM, SH], BF16, isOutput=False)
    an = nc.declare_dram_parameter("an", [N_AGENTS, SH, ADIM], F32, isOutput=False)
    wsa = nc.declare_dram_parameter("wsa", [N_AGENTS, CDIM, HID], BF16, isOutput=False)
    wse = nc.declare_dram_parameter("wse", [N_AGENTS, SDIM, HID], BF16, isOutput=False)
    wk = nc.declare_dram_parameter("wk", [HID, HID], BF16, isOutput=False)
    wsl = nc.declare_dram_parameter("wsl", [HID, HID], BF16, isOutput=False)
    wv = nc.declare_dram_parameter("wv", [HID, HID], BF16, isOutput=False)
    wc1a = nc.declare_dram_parameter("wc1a", [N_AGENTS, HID, HID], BF16, isOutput=False)
    wc1b = nc.declare_dram_parameter("wc1b", [N_AGENTS, HID, HID], BF16, isOutput=False)
    wc2 = nc.declare_dram_parameter("wc2", [N_AGENTS, HID, ADIM], BF16, isOutput=False)
    bsa = nc.declare_dram_parameter("bsa", [N_AGENTS, HID], F32, isOutput=False)
    bse = nc.declare_dram_parameter("bse", [N_AGENTS, HID], F32, isOutput=False)
    bvv = nc.declare_dram_parameter("bvv", [HID], F32, isOutput=False)
    bc1 = nc.declare_dram_parameter("bc1", [N_AGENTS, HID], F32, isOutput=False)
    bc2 = nc.declare_dram_parameter("bc2", [N_AGENTS, ADIM], F32, isOutput=False)
    cred = nc.declare_dram_parameter("cred", [HID, 32], BF16, isOutput=False)
    cz = nc.declare_dram_parameter("cz", [HID, HEADS], BF16, isOutput=False)
    crep = nc.declare_dram_parameter("crep", [HEADS, HID], F32, isOutput=False)
    cbc = nc.declare_dram_parameter("cbc", [4, HID, HID], BF16, isOutput=False)
    crow = nc.declare_dram_parameter("crow", [1, HID], BF16, isOutput=False)
    crev = nc.declare_dram_parameter("crev", [128, ADIM], F32, isOutput=False)
    out = nc.declare_dram_parameter("out", [N_AGENTS, SH], F32, isOutput=True)
    dbg = {}
    if DBG:
        for nm, shape in [("stats", [CDIM, 16]), ("san", [CDIM, BH]),
                          ("e", [HID, BH]), ("se", [HID, BH]),
                          ("keys", [HID, BH]), ("sel", [HID, BH]),
                          ("vals", [HID, BH]), ("ex", [HID, BH]),
                          ("wg", [HID, BH]), ("hh", [HID, BH]),
                          ("aq", [128, 8 * ADIM]), ("rzz", [HEADS, BH])]:
            dbg[nm] = nc.declare_dram_parameter("dbg_" + nm, shape, F32,
                                                isOutput=True)

    # internal DRAM for the BN-stats AllReduce (sum | sumsq per agent)
    stats_in = nc.dram_tensor("stats_in", [CDIM, 2 * N_AGENTS], F32)
    stats_out = nc.dram_tensor(
        "stats_out", [CDIM, 2 * N_AGENTS], F32, addr_space="Shared"
    )

    with tile.TileContext(nc) as tc:
        with (
            tc.tile_pool(name="wpool", bufs=1) as wp,     # resident weights/consts
            tc.tile_pool(name="xpool", bufs=1) as xp,     # xt tiles (phase 1+2)
            tc.tile_pool(name="stat", bufs=1) as stp,    # small per-agent vectors
            tc.tile_pool(name="work", bufs=3) as wkp,     # big transient tiles
            tc.tile_pool(name="kvpool", bufs=1) as kvp,   # keys/sel/vals/se per half
            tc.tile_pool(name="attn", bufs=2) as atp,     # exp/w tiles
            tc.tile_pool(name="qp", bufs=4) as qp,        # q path tiles
            tc.tile_pool(name="ps", bufs=3, space="PSUM") as ps,    # [128,1024] slots
            tc.tile_pool(name="wbps", bufs=2, space="PSUM") as wbp,  # [128,512] slots
        ):
            # ---------- resident weights ----------
            w_sa = [wp.tile([CDIM, HID], BF16, tag=f"wsa{n}", name=f"wsa{n}") for n in range(N_AGENTS)]
            w_se = [wp.tile([SDIM, HID], BF16, tag=f"wse{n}", name=f"wse{n}") for n in range(N_AGENTS)]
            w_k = wp.tile([HID, HID], BF16, tag="wk", name="wk")
            w_sl = wp.tile([HID, HID], BF16, tag="wsl", name="wsl")
            w_v = wp.tile([HID, HID], BF16, tag="wv", name="wv")
            w_c1a = [wp.tile([HID, HID], BF16, tag=f"wc1a{n}", name=f"wc1a{n}") for n in range(N_AGENTS)]
            w_c1b = [wp.tile([HID, HID], BF16, tag=f"wc1b{n}", name=f"wc1b{n}") for n in range(N_AGENTS)]
            w_c2 = [wp.tile([HID, ADIM], BF16, tag=f"wc2{n}", name=f"wc2{n}") for n in range(N_AGENTS)]
            b_sa = [wp.tile([HID, 1], F32, tag=f"bsa{n}", name=f"bsa{n}") for n in range(N_AGENTS)]
            b_se = [wp.tile([HID, 1], F32, tag=f"bse{n}", name=f"bse{n}") for n in range(N_AGENTS)]
            b_v = wp.tile([HID, 1], F32, tag="bv", name="bv")
            b_c1 = [wp.tile([HID, 1], F32, tag=f"bc1{n}", name=f"bc1{n}") for n in range(N_AGENTS)]
            b_c2f = [wp.tile([1, ADIM], F32, tag=f"bc2f{n}", name=f"bc2f{n}") for n in range(N_AGENTS)]
            b_c2r = [wp.tile([1, ADIM], BF16, tag=f"bc2{n}", name=f"bc2{n}") for n in range(N_AGENTS)]
            for n in range(N_AGENTS):
                nc.sync.dma_start(w_sa[n][:], wsa[n])
                nc.sync.dma_start(w_se[n][:], wse[n])
                nc.sync.dma_start(w_c1a[n][:], wc1a[n])
                nc.sync.dma_start(w_c1b[n][:], wc1b[n])
                nc.sync.dma_start(w_c2[n][:], wc2[n])
                nc.sync.dma_start(b_sa[n][:], bsa[n].unsqueeze(1))
                nc.sync.dma_start(b_se[n][:], bse[n].unsqueeze(1))
                nc.sync.dma_start(b_c1[n][:], bc1[n].unsqueeze(1))
                nc.sync.dma_start(b_c2f[n][:], bc2[n].unsqueeze(0))
                nc.vector.tensor_copy(b_c2r[n][:], b_c2f[n][:])
            nc.sync.dma_start(w_k[:], wk[:])
            nc.sync.dma_start(w_sl[:], wsl[:])
            nc.sync.dma_start(w_v[:], wv[:])
            nc.sync.dma_start(b_v[:], bvv[:].unsqueeze(1))

            # ---------- constant stationary matrices (host-built) ----------
            ones_red = wp.tile([HID, 32], BF16, tag="onesred", name="onesred")
            nc.sync.dma_start(ones_red[:], cred[:])
            ones_z = wp.tile([HID, HEADS], BF16, tag="onesz", name="onesz")
            nc.sync.dma_start(ones_z[:], cz[:])
            ones_rep = wp.tile([HEADS, HID], F32, tag="onesrep", name="onesrep")
            nc.sync.dma_start(ones_rep[:], crep[:])
            st_bc = []
            for j in range(4):
                t = wp.tile([HID, HID], BF16, tag=f"stbc{j}", name=f"stbc{j}")
                nc.sync.dma_start(t[:], cbc[j])
                st_bc.append(t)
            ones_row = wp.tile([1, HID], BF16, tag="onesrow", name="onesrow")
            nc.sync.dma_start(ones_row[:], crow[:])
            rev_f = wp.tile([128, ADIM], F32, tag="revf", name="revf")
            nc.sync.dma_start(rev_f[:], crev[:])

            # bc2 broadcast down partitions: [128, 16] = ones_row.T @ bc2_row
            bc2b = [wp.tile([128, ADIM], F32, tag=f"bc2b{n}", name=f"bc2b{n}") for n in range(N_AGENTS)]
            for n in range(N_AGENTS):
                bc2_ps = wbp.tile([128, ADIM], F32, tag="wb", name="bc2ps")
                nc.tensor.matmul(bc2_ps[:], ones_row[:], b_c2r[n][:],
                                 start=True, stop=True)
                nc.vector.tensor_copy(bc2b[n][:], bc2_ps[:])

            # ---------- phase 1: BN statistics + AllReduce ----------
            xts = []
            stats_sb = stp.tile([CDIM, 2 * N_AGENTS], F32, tag="statssb",
                                name="statssb")
            for n in range(N_AGENTS):
                xt_n = xp.tile([CDIM, SH], BF16, tag=f"xt{n}", name=f"xt{n}")
                nc.sync.dma_start(xt_n[:], xt[n])
                xts.append(xt_n)
                nc.vector.tensor_reduce(stats_sb[:, 2 * n:2 * n + 1], xt_n[:],
                                        AX.X, ALU.add)
                scr2 = wkp.tile([CDIM, SH], BF16, tag="scr", name="scr")
                nc.scalar.activation(scr2[:], xt_n[:], AF.Square,
                                     accum_out=stats_sb[:, 2 * n + 1:2 * n + 2])
            nc.sync.dma_start(stats_in[:], stats_sb[:])

            nc.gpsimd.collective_compute(
                "AllReduce", ALU.add,
                replica_groups=[list(range(N_CORES))],
                ins=[stats_in[:]],
                outs=[stats_out[:]],
            )

            # global mean/rstd per agent
            rstds, nbs = [], []
            w_sa2 = [wp.tile([CDIM, HID], BF16, tag=f"wsa2{n}", name=f"wsa2{n}")
                     for n in range(N_AGENTS)]
            w_se2 = [wp.tile([SDIM, HID], BF16, tag=f"wse2{n}", name=f"wse2{n}")
                     for n in range(N_AGENTS)]
            b_sa2 = [wp.tile([HID, 1], F32, tag=f"bsa2{n}", name=f"bsa2{n}")
                     for n in range(N_AGENTS)]
            b_se2 = [wp.tile([HID, 1], F32, tag=f"bse2{n}", name=f"bse2{n}")
                     for n in range(N_AGENTS)]
            gs_all = stp.tile([CDIM, 2 * N_AGENTS], F32, tag="gsall", name="gsall")
            nc.sync.dma_start(gs_all[:], stats_out[:])
            if DBG:
                nc.sync.dma_start(dbg["stats"][:], gs_all[:])
            for n in range(N_AGENTS):
                mean = stp.tile([CDIM, 1], F32, tag=f"mean{n}", name=f"mean{n}")
                nc.vector.tensor_scalar_mul(mean[:], gs_all[:, 2 * n:2 * n + 1],
                                            1.0 / BATCH)
                ex2 = stp.tile([CDIM, 1], F32, tag=f"ex2{n}", name=f"ex2{n}")
                nc.vector.tensor_scalar_mul(ex2[:], gs_all[:, 2 * n + 1:2 * n + 2],
                                            1.0 / BATCH)
                var = stp.tile([CDIM, 1], F32, tag=f"var{n}", name=f"var{n}")
                nc.vector.tensor_tensor(var[:], mean[:], mean[:], ALU.mult)
                nc.vector.tensor_tensor(var[:], ex2[:], var[:], ALU.subtract)
                nc.vector.tensor_scalar_add(var[:], var[:], EPS)
                ivar = stp.tile([CDIM, 1], F32, tag=f"ivar{n}", name=f"ivar{n}")
                nc.vector.reciprocal(ivar[:], var[:])
                rstd = stp.tile([CDIM, 1], F32, tag=f"rstd{n}", name=f"rstd{n}")
                nc.scalar.activation(rstd[:], ivar[:], AF.Sqrt)
                nb = stp.tile([CDIM, 1], F32, tag=f"nb{n}", name=f"nb{n}")
                nc.vector.tensor_tensor(nb[:], mean[:], rstd[:], ALU.mult)
                nc.vector.tensor_scalar_mul(nb[:], nb[:], -1.0)
                rstds.append(rstd)
                nbs.append(nb)
                # fold BN affine into the encoder weights:
                # e = lrelu(W^T((x-m)*r) + b) = lrelu((diag(r)W)^T x + (b - W^T(m*r)))
                mr = stp.tile([CDIM, 1], F32, tag=f"mr{n}", name=f"mr{n}")
                nc.vector.tensor_tensor(mr[:], mean[:], rstd[:], ALU.mult)
                mrb = stp.tile([CDIM, 1], BF16, tag=f"mrb{n}", name=f"mrb{n}")
                nc.vector.tensor_copy(mrb[:], mr[:])
                nc.vector.tensor_scalar_mul(w_sa2[n][:], w_sa[n][:], rstd[:])
                nc.vector.tensor_scalar_mul(w_se2[n][:], w_se[n][:], rstd[0:SDIM, :])
                bo_ps = wbp.tile([HID, 1], F32, tag="wb", name="bo_ps")
                nc.tensor.matmul(bo_ps[:], w_sa[n][:], mrb[:], start=True, stop=True)
                nc.vector.tensor_tensor(b_sa2[n][:], b_sa[n][:], bo_ps[:],
                                        ALU.subtract)
                bo_ps2 = wbp.tile([HID, 1], F32, tag="wb", name="bo_ps2")
                nc.tensor.matmul(bo_ps2[:], w_se[n][:], mrb[0:SDIM, :],
                                 start=True, stop=True)
                nc.vector.tensor_tensor(b_se2[n][:], b_se[n][:], bo_ps2[:],
                                        ALU.subtract)

            # ---------- phases 2-4 per batch-half ----------
            for h in range(NH):
                hs = h * BH
                # phase 2: encoders -> keys/sel/vals/se for all agents
                keys, sel, vals, se = [], [], [], []
                for n in range(N_AGENTS):
                    xv = xts[n][:, hs:hs + BH]
                    e_ps = ps.tile([HID, BH], F32, tag="ps", name="ps")
                    for c in range(NC512):
                        cs = slice(512 * c, 512 * (c + 1))
                        nc.tensor.matmul(e_ps[:, cs], w_sa2[n][:], xv[:, cs],
                                         start=True, stop=True)
                    e_n = wkp.tile([HID, BH], BF16, tag="en", name="en")
                    nc.scalar.activation(e_n[:], e_ps[:], AF.Lrelu, bias=b_sa2[n][:],
                                         alpha=SLOPE)
                    se_ps = ps.tile([HID, BH], F32, tag="ps", name="ps")
                    for c in range(NC512):
                        cs = slice(512 * c, 512 * (c + 1))
                        nc.tensor.matmul(se_ps[:, cs], w_se2[n][:],
                                         xv[0:SDIM, cs], start=True, stop=True)
                    se_n = kvp.tile([HID, BH], BF16, tag=f"se{n}", name=f"se{n}")
                    nc.scalar.activation(se_n[:], se_ps[:], AF.Lrelu, bias=b_se2[n][:],
                                         alpha=SLOPE)
                    se.append(se_n)
                    k_ps = ps.tile([HID, BH], F32, tag="ps", name="ps")
                    for c in range(NC512):
                        cs = slice(512 * c, 512 * (c + 1))
                        nc.tensor.matmul(k_ps[:, cs], w_k[:], e_n[:, cs],
                                         start=True, stop=True)
                    k_n = kvp.tile([HID, BH], BF16, tag=f"k{n}", name=f"k{n}")
                    nc.scalar.copy(k_n[:], k_ps[:])
                    keys.append(k_n)
                    sl_ps = ps.tile([HID, BH], F32, tag="ps", name="ps")
                    for c in range(NC512):
                        cs = slice(512 * c, 512 * (c + 1))
                        nc.tensor.matmul(sl_ps[:, cs], w_sl[:], se_n[:, cs],
                                         start=True, stop=True)
                    sl_n = kvp.tile([HID, BH], BF16, tag=f"sl{n}", name=f"sl{n}")
                    nc.scalar.copy(sl_n[:], sl_ps[:])
                    sel.append(sl_n)
                    v_ps = ps.tile([HID, BH], F32, tag="ps", name="ps")
                    for c in range(NC512):
                        cs = slice(512 * c, 512 * (c + 1))
                        nc.tensor.matmul(v_ps[:, cs], w_v[:], e_n[:, cs],
                                         start=True, stop=True)
                    v_n = kvp.tile([HID, BH], BF16, tag=f"v{n}", name=f"v{n}")
                    nc.scalar.activation(v_n[:], v_ps[:], AF.Lrelu, bias=b_v[:],
                                         alpha=SLOPE)
                    vals.append(v_n)
                    if DBG and h == 0 and n == 0:
                        stg = wkp.tile([HID, BH], F32, tag="dbgstg", name="dbgstg")
                        for nm, tl in [("san", sa_n), ("e", e_n), ("se", se_n),
                                       ("keys", k_n), ("sel", sl_n), ("vals", v_n)]:
                            if nm == "san":
                                nc.vector.tensor_copy(stg[0:CDIM, :], tl[:])
                                nc.sync.dma_start(dbg[nm][:], stg[0:CDIM, :])
                            else:
                                nc.vector.tensor_copy(stg[:], tl[:])
                                nc.sync.dma_start(dbg[nm][:], stg[:])
                            stg = wkp.tile([HID, BH], F32, tag="dbgstg",
                                           name="dbgstg")

                # phases 3+4: attention + critic + q, per agent i
                for i in range(N_AGENTS):
                    jall = [j for j in range(N_AGENTS) if j != i]
                    # --- logits: two [128,BH] psum tiles (j 0-3 | j 4-7), row
                    # block 32*(j%4) holds pair (i,j); diag computed then zeroed
                    lgA = ps.tile([HID, BH], F32, tag="ps", name="lgA")
                    lgB = ps.tile([HID, BH], F32, tag="ps", name="lgB")
                    for j in range(N_AGENTS):
                        if j == i:
                            continue
                        prod = wkp.tile([HID, BH], BF16, tag="prod", name="prod")
                        nc.vector.tensor_tensor(prod[:], sel[i][:], keys[j][:],
                                                ALU.mult)
                        lg = lgA if j < 4 else lgB
                        jj = j % 4
                        for c in range(NC512):
                            cs = slice(512 * c, 512 * (c + 1))
                            nc.tensor.matmul(lg[32 * jj:32 * (jj + 1), cs],
                                             ones_red[:], prod[:, cs],
                                             start=True, stop=True,
                                             tile_position=(0, 32 * jj))
                    # --- exp (scaled); diagonal row-block zeroed after ---
                    exA = atp.tile([HID, BH], BF16, tag="exA", name="exA")
                    exB = atp.tile([HID, BH], BF16, tag="exB", name="exB")
                    nc.scalar.activation(exA[:], lgA[:], AF.Exp, scale=ISQD)
                    nc.scalar.activation(exB[:], lgB[:], AF.Exp, scale=ISQD)
                    exd = exA if i < 4 else exB
                    nc.vector.memset(exd[32 * (i % 4):32 * (i % 4 + 1), :], 0.0)
                    # --- Z = sum_j exp -> [4, BH]; w = exp / Z ---
                    z_ps = ps.tile([HEADS, BH], F32, tag="ps", name="zps")
                    for c in range(NC512):
                        cs = slice(512 * c, 512 * (c + 1))
                        nc.tensor.matmul(z_ps[:, cs], ones_z[:], exA[:, cs],
                                         start=True, stop=False)
                        nc.tensor.matmul(z_ps[:, cs], ones_z[:], exB[:, cs],
                                         start=False, stop=True)
                    rz = atp.tile([HEADS, BH], F32, tag="rz", name="rz")
                    nc.vector.reciprocal(rz[:], z_ps[:])
                    rzr_ps = ps.tile([HID, BH], F32, tag="ps", name="rzrps")
                    for c in range(NC512):
                        cs = slice(512 * c, 512 * (c + 1))
                        nc.tensor.matmul(rzr_ps[:, cs], ones_rep[:], rz[:, cs],
                                         start=True, stop=True)
                    rzr_sb = atp.tile([HID, BH], BF16, tag="rzrsb", name="rzrsb")
                    nc.scalar.copy(rzr_sb[:], rzr_ps[:])
                    wgA = atp.tile([HID, BH], BF16, tag="wgA", name="wgA")
                    wgB = atp.tile([HID, BH], BF16, tag="wgB", name="wgB")
                    nc.vector.tensor_tensor(wgA[:], exA[:], rzr_sb[:], ALU.mult)
                    nc.vector.tensor_tensor(wgB[:], exB[:], rzr_sb[:], ALU.mult)
                    # --- critic h: Wc1a^T se_i + sum_j Wc1b^T (bcast(w_ij) * v_j) ---
                    h_ps = ps.tile([HID, BH], F32, tag="ps", name="h_ps")
                    for c in range(NC512):
                        cs = slice(512 * c, 512 * (c + 1))
                        nc.tensor.matmul(h_ps[:, cs], w_c1a[i][:], se[i][:, cs],
                                         start=True, stop=False)
                        for idx, j in enumerate(jall):
                            wsrc = wgA if j < 4 else wgB
                            wb_ps = wbp.tile([HID, 512], F32, tag="wb", name="wb")
                            nc.tensor.matmul(wb_ps[:], st_bc[j % 4][:],
                                             wsrc[:, cs], start=True, stop=True)
                            wv_t = wkp.tile([HID, 512], BF16, tag="wvt", name="wvt")
                            nc.vector.tensor_tensor(wv_t[:], vals[j][:, cs],
                                                    wb_ps[:], ALU.mult)
                            nc.tensor.matmul(h_ps[:, cs], w_c1b[i][:], wv_t[:],
                                             start=False, stop=(idx == len(jall) - 1))
                    h_i = wkp.tile([HID, BH], BF16, tag="hi", name="hi")
                    nc.scalar.activation(h_i[:], h_ps[:], AF.Lrelu, bias=b_c1[i][:],
                                         alpha=SLOPE)
                    if DBG and h == 0 and i == 0:
                        stg = wkp.tile([HID, BH], F32, tag="dbgstg", name="dbgstg")
                        for nm, tl in [("ex", exA), ("wg", wgA), ("hh", h_i)]:
                            nc.vector.tensor_copy(stg[:], tl[:])
                            nc.sync.dma_start(dbg[nm][:], stg[:])
                            stg = wkp.tile([HID, BH], F32, tag="dbgstg",
                                           name="dbgstg")
                        nc.vector.tensor_copy(stg[0:HEADS, :], rz[:])
                        nc.sync.dma_start(dbg["rzz"][:], stg[0:HEADS, :])
                    # --- all_q natural layout via stationary-activation matmul ---
                    aq_ps = wbp.tile([128, 8 * ADIM], F32, tag="wb", name="aq")
                    for t in range(8):  # 8 b-tiles of 128 in this half
                        nc.tensor.matmul(aq_ps[:, ADIM * t:ADIM * (t + 1)],
                                         h_i[:, 128 * t:128 * (t + 1)], w_c2[i][:],
                                         start=True, stop=True)
                    aq = qp.tile([128, 8 * ADIM], F32, tag="aqsb", name="aqsb")
                    aq3 = aq[:].rearrange("p (t k) -> p t k", t=8)
                    nc.vector.tensor_tensor(
                        aq3, aq_ps[:].rearrange("p (t k) -> p t k", t=8),
                        bc2b[i][:].unsqueeze(1).broadcast_to([128, 8, ADIM]),
                        ALU.add)
                    if DBG and h == 0 and i == 0:
                        nc.sync.dma_start(dbg["aq"][:], aq[:])
                    # --- exact argmax(a) one-hot and gather ---
                    a8 = qp.tile([128, 8 * ADIM], F32, tag="a8", name="a8")
                    nc.sync.dma_start(
                        a8[:].rearrange("p (t k) -> p t k", t=8),
                        an[i, hs:hs + BH, :].rearrange("(t p) k -> p t k", p=128))
                    a83 = a8[:].rearrange("p (t k) -> p t k", t=8)
                    amax = qp.tile([128, 8], F32, tag="amax", name="amax")
                    nc.vector.tensor_reduce(amax[:], a83, AX.X, ALU.max)
                    eq = qp.tile([128, 8 * ADIM], F32, tag="eq", name="eq")
                    eq3 = eq[:].rearrange("p (t k) -> p t k", t=8)
                    nc.vector.tensor_tensor(
                        eq3, a83, amax[:].unsqueeze(2).broadcast_to([128, 8, ADIM]),
                        ALU.is_equal)
                    nc.vector.tensor_tensor(
                        eq3, eq3, rev_f[:].unsqueeze(1).broadcast_to([128, 8, ADIM]),
                        ALU.mult)
                    smax = qp.tile([128, 8], F32, tag="smax", name="smax")
                    nc.vector.tensor_reduce(smax[:], eq3, AX.X, ALU.max)
                    nc.vector.tensor_tensor(
                        eq3, eq3, smax[:].unsqueeze(2).broadcast_to([128, 8, ADIM]),
                        ALU.is_equal)
                    nc.vector.tensor_tensor(eq3, eq3, aq3, ALU.mult)
                    q_i = qp.tile([128, 8], F32, tag="qi", name="qi")
                    nc.vector.tensor_reduce(q_i[:], eq3, AX.X, ALU.add)
                    for t in range(8):
                        nc.sync.dma_start(
                            out[i, hs + 128 * t:hs + 128 * (t + 1)].unsqueeze(1),
                            q_i[:, t:t + 1])

    nc.compile()
    return nc


def _get_nc():
    if "nc" not in _CACHE:
        _CACHE["nc"] = _build()
    return _CACHE["nc"]


def make_in_maps(s, a, W_sa, b_sa, W_se, b_se, Wk, Wsel, Wv, bv, Wc1, bc1, Wc2, bc2):
    s = np.asarray(s, np.float32)
    a = np.asarray(a, np.float32)
    x = np.concatenate([s, a], axis=-1)  # [8, 16384, 80]

    def b16(v):
        return np.ascontiguousarray(np.asarray(v, np.float32).astype(bfloat16))

    ones_red = np.zeros((HID, 32), np.float32)
    ones_z = np.zeros((HID, HEADS), np.float32)
    ones_rep = np.zeros((HEADS, HID), np.float32)
    st_bc = np.zeros((4, HID, HID), np.float32)
    for p in range(HEADS):
        ones_red[32 * p:32 * (p + 1), 8 * p:8 * (p + 1)] = 1.0
        for j in range(4):
            ones_z[32 * j + 8 * p, p] = 1.0
            ones_rep[p, 32 * j + 8 * p:32 * j + 8 * p + 8] = 1.0
            st_bc[j, 32 * j + 8 * p, 32 * p:32 * (p + 1)] = 1.0
    rev = np.tile(np.arange(ADIM, 0, -1, dtype=np.float32), (128, 1))
    shared = {
        "cred": b16(ones_red), "cz": b16(ones_z),
        "crep": np.ascontiguousarray(ones_rep),
        "cbc": b16(st_bc), "crow": b16(np.ones((1, HID), np.float32)),
        "crev": np.ascontiguousarray(rev),
        "wsa": b16(W_sa), "wse": b16(W_se),
        "wk": b16(np.transpose(np.asarray(Wk, np.float32), (1, 0, 2)).reshape(HID, HID)),
        "wsl": b16(np.transpose(np.asarray(Wsel, np.float32), (1, 0, 2)).reshape(HID, HID)),
        "wv": b16(np.transpose(np.asarray(Wv, np.float32), (1, 0, 2)).reshape(HID, HID)),
        "wc1a": b16(np.asarray(Wc1, np.float32)[:, :HID, :]),
        "wc1b": b16(np.asarray(Wc1, np.float32)[:, HID:, :]),
        "wc2": b16(Wc2),
        "bsa": np.ascontiguousarray(np.asarray(b_sa, np.float32)),
        "bse": np.ascontiguousarray(np.asarray(b_se, np.float32)),
        "bvv": np.ascontiguousarray(np.asarray(bv, np.float32).reshape(HID)),
        "bc1": np.ascontiguousarray(np.asarray(bc1, np.float32)),
        "bc2": np.ascontiguousarray(np.asarray(bc2, np.float32)),
    }
    in_maps = []
    for c in range(N_CORES):
        sl = slice(c * SH, (c + 1) * SH)
        m = dict(shared)
        m["xt"] = np.ascontiguousarray(
            np.transpose(x[:, sl, :], (0, 2, 1)).astype(np.float32)).astype(bfloat16)
        m["an"] = np.ascontiguousarray(a[:, sl, :])
        in_maps.append(m)
    return in_maps


def kernel(**inputs):
    in_maps = make_in_maps(**inputs)
    nc = _get_nc()
    res = run_bass_kernel_spmd(nc, in_maps, core_ids=list(range(N_CORES)))
    outs = [np.asarray(res.results[c]["out"]) for c in range(N_CORES)]
    q = np.concatenate(outs, axis=1)  # [8, 16384]
    return q[..., None].astype(np.float32)


if __name__ == "__main__":
    import reference as R
    inp = {k: np.asarray(v) for k, v in R.setup_inputs().items()}
    got = kernel(**inp)
    print("kernel out", got.shape)



# revision 2
# speedup vs baseline: 612.3818x; 612.3818x over previous
"""Trainium2 Bass kernel for nn_Attention_Critic (8-agent attention critic).

Data-parallel over batch across 8 NeuronCores (2048 batch rows per core).
BatchNorm statistics are made global via a tiny AllReduce (8x160 f32).

Device layout: feature-on-partitions, [feat, batch] tiles.
  - host pre-transposes s|a to [agent, 80, 2048] per core (bf16)
  - encoders / projections: PE matmuls, contraction on partitions
  - attention einsum pibd,pjbd->pijb: DVE pairwise muls on [128=(head,d), B]
    tiles + PE block-ones matmuls reducing over d (partition reduce) into a
    compact [32=(4j+p), B] logits block
  - softmax: ACT exp, PE ones-matmul for the j-sum, DVE reciprocal; weights
    broadcast d-wise back to [128, B] by a second ones-pattern matmul
  - weighted sum over agents folded into the critic's PSUM accumulation
  - argmax-gather of q: exact first-max logic with DVE compare/reduce ops
"""

import sys

sys.path.insert(0, "/opt/trn_rl_repo")

import numpy as np
from ml_dtypes import bfloat16

from concourse import bacc, bass, mybir, tile
from concourse.bass_utils import run_bass_kernel_spmd

F32 = mybir.dt.float32
BF16 = mybir.dt.bfloat16
AF = mybir.ActivationFunctionType
ALU = mybir.AluOpType
AX = mybir.AxisListType

N_AGENTS = 8
BATCH = 16384
SDIM, ADIM = 64, 16
CDIM = SDIM + ADIM  # 80
HID = 128
HEADS = 4
ATT_D = 32
N_CORES = 8
SH = BATCH // N_CORES  # 2048 batch rows per core
NH = 2  # batch halves per core
BH = SH // NH  # 1024
NC512 = BH // 512  # matmul chunks per half
EPS = 1e-5
ISQD = float(1.0 / np.sqrt(np.float32(ATT_D)))
SLOPE = 0.01  # LeakyReLU negative slope

_CACHE = {}
DBG = False


def _build():
    nc = bacc.Bacc(None, num_devices=N_CORES)

    # ---- DRAM parameters (per-core shard shapes) ----
    xt = nc.declare_dram_parameter("xt", [N_AGENTS, CDIM, SH], BF16, isOutput=False)
    an = nc.declare_dram_parameter("an", [N_AGENTS, SH, ADIM], F32, isOutput=False)
    wsa = nc.declare_dram_parameter("wsa", [N_AGENTS, CDIM, HID], BF16, isOutput=False)
    wse = nc.declare_dram_parameter("wse", [N_AGENTS, SDIM, HID], BF16, isOutput=False)
    wk = nc.declare_dram_parameter("wk", [HID, HID], BF16, isOutput=False)
    wsl = nc.declare_dram_parameter("wsl", [HID, HID], BF16, isOutput=False)
    wv = nc.declare_dram_parameter("wv", [HID, HID], BF16, isOutput=False)
    wc1a = nc.declare_dram_parameter("wc1a", [N_AGENTS, HID, HID], BF16, isOutput=False)
    wc1b = nc.declare_dram_parameter("wc1b", [N_AGENTS, HID, HID], BF16, isOutput=False)
    wc2 = nc.declare_dram_parameter("wc2", [N_AGENTS, HID, ADIM], BF16, isOutput=False)
    bsa = nc.declare_dram_parameter("bsa", [N_AGENTS, HID], F32, isOutput=False)
    bse = nc.declare_dram_parameter("bse", [N_AGENTS, HID], F32, isOutput=False)
    bvv = nc.declare_dram_parameter("bvv", [HID], F32, isOutput=False)
    bc1 = nc.declare_dram_parameter("bc1", [N_AGENTS, HID], F32, isOutput=False)
    bc2 = nc.declare_dram_parameter("bc2", [N_AGENTS, ADIM], F32, isOutput=False)
    cred = nc.declare_dram_parameter("cred", [HID, 32], BF16, isOutput=False)
    cz = nc.declare_dram_parameter("cz", [HID, HEADS], BF16, isOutput=False)
    crep = nc.declare_dram_parameter("crep", [HEADS, HID], F32, isOutput=False)
    cbc = nc.declare_dram_parameter("cbc", [4, HID, HID], BF16, isOutput=False)
    crow = nc.declare_dram_parameter("crow", [1, HID], BF16, isOutput=False)
    crev = nc.declare_dram_parameter("crev", [128, ADIM], F32, isOutput=False)
    out = nc.declare_dram_parameter("out", [N_AGENTS, SH], F32, isOutput=True)
    dbg = {}
    if DBG:
        for nm, shape in [("stats", [CDIM, 16]), ("san", [CDIM, BH]),
                          ("e", [HID, BH]), ("se", [HID, BH]),
                          ("keys", [HID, BH]), ("sel", [HID, BH]),
                          ("vals", [HID, BH]), ("ex", [HID, BH]),
                          ("wg", [HID, BH]), ("hh", [HID, BH]),
                          ("aq", [128, 8 * ADIM]), ("rzz", [HEADS, BH])]:
            dbg[nm] = nc.declare_dram_parameter("dbg_" + nm, shape, F32,
                                                isOutput=True)

    # internal DRAM for the BN-stats AllReduce (sum | sumsq per agent)
    stats_in = nc.dram_tensor("stats_in", [CDIM, 2 * N_AGENTS], F32)
    stats_out = nc.dram_tensor(
        "stats_out", [CDIM, 2 * N_AGENTS], F32, addr_space="Shared"
    )

    with tile.TileContext(nc) as tc:
        with (
            tc.tile_pool(name="wpool", bufs=1) as wp,     # resident weights/consts
            tc.tile_pool(name="xpool", bufs=1) as xp,     # xt tiles (phase 1+2)
            tc.tile_pool(name="stat", bufs=1) as stp,    # small per-agent vectors
            tc.tile_pool(name="work", bufs=3) as wkp,     # big transient tiles
            tc.tile_pool(name="kvpool", bufs=1) as kvp,   # keys/sel/vals/se per half
            tc.tile_pool(name="attn", bufs=2) as atp,     # exp/w tiles
            tc.tile_pool(name="qp", bufs=4) as qp,        # q path tiles
            tc.tile_pool(name="ps", bufs=3, space="PSUM") as ps,    # [128,1024] slots
            tc.tile_pool(name="wbps", bufs=2, space="PSUM") as wbp,  # [128,512] slots
        ):
            # ---------- resident weights ----------
            w_sa = [wp.tile([CDIM, HID], BF16, tag=f"wsa{n}", name=f"wsa{n}") for n in range(N_AGENTS)]
            w_se = [wp.tile([SDIM, HID], BF16, tag=f"wse{n}", name=f"wse{n}") for n in range(N_AGENTS)]
            w_k = wp.tile([HID, HID], BF16, tag="wk", name="wk")
            w_sl = wp.tile([HID, HID], BF16, tag="wsl", name="wsl")
            w_v = wp.tile([HID, HID], BF16, tag="wv", name="wv")
            w_c1a = [wp.tile([HID, HID], BF16, tag=f"wc1a{n}", name=f"wc1a{n}") for n in range(N_AGENTS)]
            w_c1b = [wp.tile([HID, HID], BF16, tag=f"wc1b{n}", name=f"wc1b{n}") for n in range(N_AGENTS)]
            w_c2 = [wp.tile([HID, ADIM], BF16, tag=f"wc2{n}", name=f"wc2{n}") for n in range(N_AGENTS)]
            b_sa = [wp.tile([HID, 1], F32, tag=f"bsa{n}", name=f"bsa{n}") for n in range(N_AGENTS)]
            b_se = [wp.tile([HID, 1], F32, tag=f"bse{n}", name=f"bse{n}") for n in range(N_AGENTS)]
            b_v = wp.tile([HID, 1], F32, tag="bv", name="bv")
            b_c1 = [wp.tile([HID, 1], F32, tag=f"bc1{n}", name=f"bc1{n}") for n in range(N_AGENTS)]
            b_c2f = [wp.tile([1, ADIM], F32, tag=f"bc2f{n}", name=f"bc2f{n}") for n in range(N_AGENTS)]
            b_c2r = [wp.tile([1, ADIM], BF16, tag=f"bc2{n}", name=f"bc2{n}") for n in range(N_AGENTS)]
            for n in range(N_AGENTS):
                nc.sync.dma_start(w_sa[n][:], wsa[n])
                nc.sync.dma_start(w_se[n][:], wse[n])
                nc.sync.dma_start(w_c1a[n][:], wc1a[n])
                nc.sync.dma_start(w_c1b[n][:], wc1b[n])
                nc.sync.dma_start(w_c2[n][:], wc2[n])
                nc.sync.dma_start(b_sa[n][:], bsa[n].unsqueeze(1))
                nc.sync.dma_start(b_se[n][:], bse[n].unsqueeze(1))
                nc.sync.dma_start(b_c1[n][:], bc1[n].unsqueeze(1))
                nc.sync.dma_start(b_c2f[n][:], bc2[n].unsqueeze(0))
                nc.vector.tensor_copy(b_c2r[n][:], b_c2f[n][:])
            nc.sync.dma_start(w_k[:], wk[:])
            nc.sync.dma_start(w_sl[:], wsl[:])
            nc.sync.dma_start(w_v[:], wv[:])
            nc.sync.dma_start(b_v[:], bvv[:].unsqueeze(1))

            # ---------- constant stationary matrices (host-built) ----------
            ones_red = wp.tile([HID, 32], BF16, tag="onesred", name="onesred")
            nc.sync.dma_start(ones_red[:], cred[:])
            ones_z = wp.tile([HID, HEADS], BF16, tag="onesz", name="onesz")
            nc.sync.dma_start(ones_z[:], cz[:])
            ones_rep = wp.tile([HEADS, HID], F32, tag="onesrep", name="onesrep")
            nc.sync.dma_start(ones_rep[:], crep[:])
            st_bc = []
            for j in range(4):
                t = wp.tile([HID, HID], BF16, tag=f"stbc{j}", name=f"stbc{j}")
                nc.sync.dma_start(t[:], cbc[j])
                st_bc.append(t)
            ones_row = wp.tile([1, HID], BF16, tag="onesrow", name="onesrow")
            nc.sync.dma_start(ones_row[:], crow[:])
            rev_f = wp.tile([128, ADIM], F32, tag="revf", name="revf")
            nc.sync.dma_start(rev_f[:], crev[:])

            # bc2 broadcast down partitions: [128, 16] = ones_row.T @ bc2_row
            bc2b = [wp.tile([128, ADIM], F32, tag=f"bc2b{n}", name=f"bc2b{n}") for n in range(N_AGENTS)]
            for n in range(N_AGENTS):
                bc2_ps = wbp.tile([128, ADIM], F32, tag="wb", name="bc2ps")
                nc.tensor.matmul(bc2_ps[:], ones_row[:], b_c2r[n][:],
                                 start=True, stop=True)
                nc.vector.tensor_copy(bc2b[n][:], bc2_ps[:])

            # ---------- phase 1: BN statistics + AllReduce ----------
            xts = []
            stats_sb = stp.tile([CDIM, 2 * N_AGENTS], F32, tag="statssb",
                                name="statssb")
            for n in range(N_AGENTS):
                xt_n = xp.tile([CDIM, SH], BF16, tag=f"xt{n}", name=f"xt{n}")
                nc.sync.dma_start(xt_n[:], xt[n])
                xts.append(xt_n)
                nc.vector.tensor_reduce(stats_sb[:, 2 * n:2 * n + 1], xt_n[:],
                                        AX.X, ALU.add)
                scr2 = wkp.tile([CDIM, SH], BF16, tag="scr", name="scr")
                nc.scalar.activation(scr2[:], xt_n[:], AF.Square,
                                     accum_out=stats_sb[:, 2 * n + 1:2 * n + 2])
            nc.sync.dma_start(stats_in[:], stats_sb[:])

            nc.gpsimd.collective_compute(
                "AllReduce", ALU.add,
                replica_groups=[list(range(N_CORES))],
                ins=[stats_in[:]],
                outs=[stats_out[:]],
            )

            # global mean/rstd per agent
            rstds, nbs = [], []
            w_sa2 = [wp.tile([CDIM, HID], BF16, tag=f"wsa2{n}", name=f"wsa2{n}")
                     for n in range(N_AGENTS)]
            w_se2 = [wp.tile([SDIM, HID], BF16, tag=f"wse2{n}", name=f"wse2{n}")
                     for n in range(N_AGENTS)]
            b_sa2 = [wp.tile([HID, 1], F32, tag=f"bsa2{n}", name=f"bsa2{n}")
                     for n in range(N_AGENTS)]
            b_se2 = [wp.tile([HID, 1], F32, tag=f"bse2{n}", name=f"bse2{n}")
                     for n in range(N_AGENTS)]
            gs_all = stp.tile([CDIM, 2 * N_AGENTS], F32, tag="gsall", name="gsall")
            nc.sync.dma_start(gs_all[:], stats_out[:])
            if DBG:
                nc.sync.dma_start(dbg["stats"][:], gs_all[:])
            for n in range(N_AGENTS):
                mean = stp.tile([CDIM, 1], F32, tag=f"mean{n}", name=f"mean{n}")
                nc.vector.tensor_scalar_mul(mean[:], gs_all[:, 2 * n:2 * n + 1],
                                            1.0 / BATCH)
                ex2 = stp.tile([CDIM, 1], F32, tag=f"ex2{n}", name=f"ex2{n}")
                nc.vector.tensor_scalar_mul(ex2[:], gs_all[:, 2 * n + 1:2 * n + 2],
                                            1.0 / BATCH)
                var = stp.tile([CDIM, 1], F32, tag=f"var{n}", name=f"var{n}")
                nc.vector.tensor_tensor(var[:], mean[:], mean[:], ALU.mult)
                nc.vector.tensor_tensor(var[:], ex2[:], var[:], ALU.subtract)
                nc.vector.tensor_scalar_add(var[:], var[:], EPS)
                ivar = stp.tile([CDIM, 1], F32, tag=f"ivar{n}", name=f"ivar{n}")
                nc.vector.reciprocal(ivar[:], var[:])
                rstd = stp.tile([CDIM, 1], F32, tag=f"rstd{n}", name=f"rstd{n}")
                nc.scalar.activation(rstd[:], ivar[:], AF.Sqrt)
                nb = stp.tile([CDIM, 1], F32, tag=f"nb{n}", name=f"nb{n}")
                nc.vector.tensor_tensor(nb[:], mean[:], rstd[:], ALU.mult)
                nc.vector.tensor_scalar_mul(nb[:], nb[:], -1.0)
                rstds.append(rstd)
                nbs.append(nb)
                # fold BN affine into the encoder weights:
                # e = lrelu(W^T((x-m)*r) + b) = lrelu((diag(r)W)^T x + (b - W^T(m*r)))
                mr = stp.tile([CDIM, 1], F32, tag=f"mr{n}", name=f"mr{n}")
                nc.vector.tensor_tensor(mr[:], mean[:], rstd[:], ALU.mult)
                mrb = stp.tile([CDIM, 1], BF16, tag=f"mrb{n}", name=f"mrb{n}")
                nc.vector.tensor_copy(mrb[:], mr[:])
                nc.vector.tensor_scalar_mul(w_sa2[n][:], w_sa[n][:], rstd[:])
                nc.vector.tensor_scalar_mul(w_se2[n][:], w_se[n][:], rstd[0:SDIM, :])
                bo_ps = wbp.tile([HID, 1], F32, tag="wb", name="bo_ps")
                nc.tensor.matmul(bo_ps[:], w_sa[n][:], mrb[:], start=True, stop=True)
                nc.vector.tensor_tensor(b_sa2[n][:], b_sa[n][:], bo_ps[:],
                                        ALU.subtract)
                bo_ps2 = wbp.tile([HID, 1], F32, tag="wb", name="bo_ps2")
                nc.tensor.matmul(bo_ps2[:], w_se[n][:], mrb[0:SDIM, :],
                                 start=True, stop=True)
                nc.vector.tensor_tensor(b_se2[n][:], b_se[n][:], bo_ps2[:],
                                        ALU.subtract)

            # ---------- phases 2-4 per batch-half ----------
            for h in range(NH):
                hs = h * BH
                # phase 2: encoders -> keys/sel/vals/se for all agents
                keys, sel, vals, se = [], [], [], []
                for n in range(N_AGENTS):
                    xv = xts[n][:, hs:hs + BH]
                    e_ps = ps.tile([HID, BH], F32, tag="ps", name="ps")
                    for c in range(NC512):
                        cs = slice(512 * c, 512 * (c + 1))
                        nc.tensor.matmul(e_ps[:, cs], w_sa2[n][:], xv[:, cs],
                                         start=True, stop=True)
                    e_n = wkp.tile([HID, BH], BF16, tag="en", name="en")
                    nc.scalar.activation(e_n[:], e_ps[:], AF.Lrelu, bias=b_sa2[n][:],
                                         alpha=SLOPE)
                    se_ps = ps.tile([HID, BH], F32, tag="ps", name="ps")
                    for c in range(NC512):
                        cs = slice(512 * c, 512 * (c + 1))
                        nc.tensor.matmul(se_ps[:, cs], w_se2[n][:],
                                         xv[0:SDIM, cs], start=True, stop=True)
                    se_n = kvp.tile([HID, BH], BF16, tag=f"se{n}", name=f"se{n}")
                    nc.scalar.activation(se_n[:], se_ps[:], AF.Lrelu, bias=b_se2[n][:],
                                         alpha=SLOPE)
                    se.append(se_n)
                    k_ps = ps.tile([HID, BH], F32, tag="ps", name="ps")
                    for c in range(NC512):
                        cs = slice(512 * c, 512 * (c + 1))
                        nc.tensor.matmul(k_ps[:, cs], w_k[:], e_n[:, cs],
                                         start=True, stop=True)
                    k_n = kvp.tile([HID, BH], BF16, tag=f"k{n}", name=f"k{n}")
                    nc.scalar.copy(k_n[:], k_ps[:])
                    keys.append(k_n)
                    sl_ps = ps.tile([HID, BH], F32, tag="ps", name="ps")
                    for c in range(NC512):
                        cs = slice(512 * c, 512 * (c + 1))
                        nc.tensor.matmul(sl_ps[:, cs], w_sl[:], se_n[:, cs],
                                         start=True, stop=True)
                    sl_n = kvp.tile([HID, BH], BF16, tag=f"sl{n}", name=f"sl{n}")
                    nc.scalar.copy(sl_n[:], sl_ps[:])
                    sel.append(sl_n)
                    v_ps = ps.tile([HID, BH], F32, tag="ps", name="ps")
                    for c in range(NC512):
                        cs = slice(512 * c, 512 * (c + 1))
                        nc.tensor.matmul(v_ps[:, cs], w_v[:], e_n[:, cs],
                                         start=True, stop=True)
                    v_n = kvp.tile([HID, BH], BF16, tag=f"v{n}", name=f"v{n}")
                    nc.scalar.activation(v_n[:], v_ps[:], AF.Lrelu, bias=b_v[:],
                                         alpha=SLOPE)
                    vals.append(v_n)
                    if DBG and h == 0 and n == 0:
                        stg = wkp.tile([HID, BH], F32, tag="dbgstg", name="dbgstg")
                        for nm, tl in [("san", sa_n), ("e", e_n), ("se", se_n),
                                       ("keys", k_n), ("sel", sl_n), ("vals", v_n)]:
                            if nm == "san":
                                nc.vector.tensor_copy(stg[0:CDIM, :], tl[:])
                                nc.sync.dma_start(dbg[nm][:], stg[0:CDIM, :])
                            else:
                                nc.vector.tensor_copy(stg[:], tl[:])
                                nc.sync.dma_start(dbg[nm][:], stg[:])
                            stg = wkp.tile([HID, BH], F32, tag="dbgstg",
                                           name="dbgstg")

                # phases 3+4: attention + critic + q, per agent i
                for i in range(N_AGENTS):
                    jall = [j for j in range(N_AGENTS) if j != i]
                    # --- logits: two [128,BH] psum tiles (j 0-3 | j 4-7), row
                    # block 32*(j%4) holds pair (i,j); diag computed then zeroed
                    lgA = ps.tile([HID, BH], F32, tag="ps", name="lgA")
                    lgB = ps.tile([HID, BH], F32, tag="ps", name="lgB")
                    for j in range(N_AGENTS):
                        if j == i:
                            continue
                        prod = wkp.tile([HID, BH], BF16, tag="prod", name="prod")
                        nc.vector.tensor_tensor(prod[:], sel[i][:], keys[j][:],
                                                ALU.mult)
                        lg = lgA if j < 4 else lgB
                        jj = j % 4
                        for c in range(NC512):
                            cs = slice(512 * c, 512 * (c + 1))
                            nc.tensor.matmul(lg[32 * jj:32 * (jj + 1), cs],
                                             ones_red[:], prod[:, cs],
                                             start=True, stop=True,
                                             tile_position=(0, 32 * jj))
                    # --- exp (scaled); diagonal row-block zeroed after ---
                    exA = atp.tile([HID, BH], BF16, tag="exA", name="exA")
                    exB = atp.tile([HID, BH], BF16, tag="exB", name="exB")
                    nc.scalar.activation(exA[:], lgA[:], AF.Exp, scale=ISQD)
                    nc.scalar.activation(exB[:], lgB[:], AF.Exp, scale=ISQD)
                    exd = exA if i < 4 else exB
                    nc.vector.memset(exd[32 * (i % 4):32 * (i % 4 + 1), :], 0.0)
                    # --- Z = sum_j exp -> [4, BH]; w = exp / Z ---
                    z_ps = ps.tile([HEADS, BH], F32, tag="ps", name="zps")
                    for c in range(NC512):
                        cs = slice(512 * c, 512 * (c + 1))
                        nc.tensor.matmul(z_ps[:, cs], ones_z[:], exA[:, cs],
                                         start=True, stop=False)
                        nc.tensor.matmul(z_ps[:, cs], ones_z[:], exB[:, cs],
                                         start=False, stop=True)
                    rz = atp.tile([HEADS, BH], F32, tag="rz", name="rz")
                    nc.vector.reciprocal(rz[:], z_ps[:])
                    rzr_ps = ps.tile([HID, BH], F32, tag="ps", name="rzrps")
                    for c in range(NC512):
                        cs = slice(512 * c, 512 * (c + 1))
                        nc.tensor.matmul(rzr_ps[:, cs], ones_rep[:], rz[:, cs],
                                         start=True, stop=True)
                    rzr_sb = atp.tile([HID, BH], BF16, tag="rzrsb", name="rzrsb")
                    nc.scalar.copy(rzr_sb[:], rzr_ps[:])
                    wgA = atp.tile([HID, BH], BF16, tag="wgA", name="wgA")
                    wgB = atp.tile([HID, BH], BF16, tag="wgB", name="wgB")
                    nc.vector.tensor_tensor(wgA[:], exA[:], rzr_sb[:], ALU.mult)
                    nc.vector.tensor_tensor(wgB[:], exB[:], rzr_sb[:], ALU.mult)
                    # --- critic h: Wc1a^T se_i + sum_j Wc1b^T (bcast(w_ij) * v_j) ---
                    h_ps = ps.tile([HID, BH], F32, tag="ps", name="h_ps")
                    for c in range(NC512):
                        cs = slice(512 * c, 512 * (c + 1))
                        nc.tensor.matmul(h_ps[:, cs], w_c1a[i][:], se[i][:, cs],
                                         start=True, stop=False)
                        for idx, j in enumerate(jall):
                            wsrc = wgA if j < 4 else wgB
                            wb_ps = wbp.tile([HID, 512], F32, tag="wb", name="wb")
                            nc.tensor.matmul(wb_ps[:], st_bc[j % 4][:],
                                             wsrc[:, cs], start=True, stop=True)
                            wv_t = wkp.tile([HID, 512], BF16, tag="wvt", name="wvt")
                            nc.vector.tensor_tensor(wv_t[:], vals[j][:, cs],
                                                    wb_ps[:], ALU.mult)
                            nc.tensor.matmul(h_ps[:, cs], w_c1b[i][:], wv_t[:],
                                             start=False, stop=(idx == len(jall) - 1))
                    h_i = wkp.tile([HID, BH], BF16, tag="hi", name="hi")
                    nc.scalar.activation(h_i[:], h_ps[:], AF.Lrelu, bias=b_c1[i][:],
                                         alpha=SLOPE)
                    if DBG and h == 0 and i == 0:
                        stg = wkp.tile([HID, BH], F32, tag="dbgstg", name="dbgstg")
                        for nm, tl in [("ex", exA), ("wg", wgA), ("hh", h_i)]:
                            nc.vector.tensor_copy(stg[:], tl[:])
                            nc.sync.dma_start(dbg[nm][:], stg[:])
                            stg = wkp.tile([HID, BH], F32, tag="dbgstg",
                                           name="dbgstg")
                        nc.vector.tensor_copy(stg[0:HEADS, :], rz[:])
                        nc.sync.dma_start(dbg["rzz"][:], stg[0:HEADS, :])
                    # --- all_q natural layout via stationary-activation matmul ---
                    aq_ps = wbp.tile([128, 8 * ADIM], F32, tag="wb", name="aq")
                    for t in range(8):  # 8 b-tiles of 128 in this half
                        nc.tensor.matmul(aq_ps[:, ADIM * t:ADIM * (t + 1)],
                                         h_i[:, 128 * t:128 * (t + 1)], w_c2[i][:],
                                         start=True, stop=True)
                    aq = qp.tile([128, 8 * ADIM], F32, tag="aqsb", name="aqsb")
                    aq3 = aq[:].rearrange("p (t k) -> p t k", t=8)
                    nc.vector.tensor_tensor(
                        aq3, aq_ps[:].rearrange("p (t k) -> p t k", t=8),
                        bc2b[i][:].unsqueeze(1).broadcast_to([128, 8, ADIM]),
                        ALU.add)
                    if DBG and h == 0 and i == 0:
                        nc.sync.dma_start(dbg["aq"][:], aq[:])
                    # --- exact argmax(a) one-hot and gather ---
                    a8 = qp.tile([128, 8 * ADIM], F32, tag="a8", name="a8")
                    nc.sync.dma_start(
                        a8[:].rearrange("p (t k) -> p t k", t=8),
                        an[i, hs:hs + BH, :].rearrange("(t p) k -> p t k", p=128))
                    a83 = a8[:].rearrange("p (t k) -> p t k", t=8)
                    amax = qp.tile([128, 8], F32, tag="amax", name="amax")
                    nc.vector.tensor_reduce(amax[:], a83, AX.X, ALU.max)
                    eq = qp.tile([128, 8 * ADIM], F32, tag="eq", name="eq")
                    eq3 = eq[:].rearrange("p (t k) -> p t k", t=8)
                    nc.vector.tensor_tensor(
                        eq3, a83, amax[:].unsqueeze(2).broadcast_to([128, 8, ADIM]),
                        ALU.is_equal)
                    nc.vector.tensor_tensor(
                        eq3, eq3, rev_f[:].unsqueeze(1).broadcast_to([128, 8, ADIM]),
                        ALU.mult)
                    smax = qp.tile([128, 8], F32, tag="smax", name="smax")
                    nc.vector.tensor_reduce(smax[:], eq3, AX.X, ALU.max)
                    nc.vector.tensor_tensor(
                        eq3, eq3, smax[:].unsqueeze(2).broadcast_to([128, 8, ADIM]),
                        ALU.is_equal)
                    nc.vector.tensor_tensor(eq3, eq3, aq3, ALU.mult)
                    q_i = qp.tile([128, 8], F32, tag="qi", name="qi")
                    nc.vector.tensor_reduce(q_i[:], eq3, AX.X, ALU.add)
                    for t in range(8):
                        nc.sync.dma_start(
                            out[i, hs + 128 * t:hs + 128 * (t + 1)].unsqueeze(1),
                            q_i[:, t:t + 1])

    nc.compile()
    return nc


def _get_nc():
    if "nc" not in _CACHE:
        _CACHE["nc"] = _build()
    return _CACHE["nc"]


def make_in_maps(s, a, W_sa, b_sa, W_se, b_se, Wk, Wsel, Wv, bv, Wc1, bc1, Wc2, bc2):
    s = np.asarray(s, np.float32)
    a = np.asarray(a, np.float32)
    x = np.concatenate([s, a], axis=-1)  # [8, 16384, 80]

    def b16(v):
        return np.ascontiguousarray(np.asarray(v, np.float32).astype(bfloat16))

    ones_red = np.zeros((HID, 32), np.float32)
    ones_z = np.zeros((HID, HEADS), np.float32)
    ones_rep = np.zeros((HEADS, HID), np.float32)
    st_bc = np.zeros((4, HID, HID), np.float32)
    for p in range(HEADS):
        ones_red[32 * p:32 * (p + 1), 8 * p:8 * (p + 1)] = 1.0
        for j in range(4):
            ones_z[32 * j + 8 * p, p] = 1.0
            ones_rep[p, 32 * j + 8 * p:32 * j + 8 * p + 8] = 1.0
            st_bc[j, 32 * j + 8 * p, 32 * p:32 * (p + 1)] = 1.0
    rev = np.tile(np.arange(ADIM, 0, -1, dtype=np.float32), (128, 1))
    shared = {
        "cred": b16(ones_red), "cz": b16(ones_z),
        "crep": np.ascontiguousarray(ones_rep),
        "cbc": b16(st_bc), "crow": b16(np.ones((1, HID), np.float32)),
        "crev": np.ascontiguousarray(rev),
        "wsa": b16(W_sa), "wse": b16(W_se),
        "wk": b16(np.transpose(np.asarray(Wk, np.float32), (1, 0, 2)).reshape(HID, HID)),
        "wsl": b16(np.transpose(np.asarray(Wsel, np.float32), (1, 0, 2)).reshape(HID, HID)),
        "wv": b16(np.transpose(np.asarray(Wv, np.float32), (1, 0, 2)).reshape(HID, HID)),
        "wc1a": b16(np.asarray(Wc1, np.float32)[:, :HID, :]),
        "wc1b": b16(np.asarray(Wc1, np.float32)[:, HID:, :]),
        "wc2": b16(Wc2),
        "bsa": np.ascontiguousarray(np.asarray(b_sa, np.float32)),
        "bse": np.ascontiguousarray(np.asarray(b_se, np.float32)),
        "bvv": np.ascontiguousarray(np.asarray(bv, np.float32).reshape(HID)),
        "bc1": np.ascontiguousarray(np.asarray(bc1, np.float32)),
        "bc2": np.ascontiguousarray(np.asarray(bc2, np.float32)),
    }
    in_maps = []
    for c in range(N_CORES):
        sl = slice(c * SH, (c + 1) * SH)
        m = dict(shared)
        m["xt"] = np.ascontiguousarray(
            np.transpose(x[:, sl, :], (0, 2, 1)).astype(np.float32)).astype(bfloat16)
        m["an"] = np.ascontiguousarray(a[:, sl, :])
        in_maps.append(m)
    return in_maps


def kernel(**inputs):
    in_maps = make_in_maps(**inputs)
    nc = _get_nc()
    res = run_bass_kernel_spmd(nc, in_maps, core_ids=list(range(N_CORES)))
    outs = [np.asarray(res.results[c]["out"]) for c in range(N_CORES)]
    q = np.concatenate(outs, axis=1)  # [8, 16384]
    return q[..., None].astype(np.float32)


def make_runner(in_maps):
    """Build the sharded PJRT executable ONCE and pre-stage the per-core
    inputs on the 8 devices. Returns (run_n, fetch) where run_n(n)
    dispatches n back-to-back executions of the NEFF and blocks until all
    complete, and fetch() returns the [8,16384,1] output of the last run.

    This is the measurement path: executable reuse + device-resident
    inputs isolate NEFF execution from per-call jit re-tracing, BIR
    recompilation and host->device staging that run_bass_kernel_spmd
    re-pays on every invocation.
    """
    import jax
    from jax.sharding import Mesh, NamedSharding, PartitionSpec
    from jax.experimental.shard_map import shard_map
    from concourse.bass2jax import (
        _bass_exec_p, partition_id_tensor, install_neuronx_cc_hook)

    install_neuronx_cc_hook()
    nc = _get_nc()
    partition_name = (nc.partition_id_tensor.name
                      if nc.partition_id_tensor else None)
    in_names, out_names, out_avals, zero_outs = [], [], [], []
    for alloc in nc.m.functions[0].allocations:
        if not isinstance(alloc, mybir.MemoryLocationSet):
            continue
        name = alloc.memorylocations[0].name
        if alloc.kind == "ExternalInput":
            if name != partition_name:
                in_names.append(name)
        elif alloc.kind == "ExternalOutput":
            shape = tuple(alloc.tensor_shape)
            dtype = mybir.dt.np(alloc.dtype)
            out_names.append(name)
            out_avals.append(jax.core.ShapedArray(shape, dtype))
            zero_outs.append(np.zeros((N_CORES * shape[0], *shape[1:]), dtype))
    in_names_all = in_names + out_names + (
        [partition_name] if partition_name else [])

    def _body(*args):
        operands = list(args)
        if partition_name is not None:
            operands.append(partition_id_tensor())
        return tuple(_bass_exec_p.bind(
            *operands, out_avals=tuple(out_avals), in_names=tuple(in_names_all),
            out_names=tuple(out_names), lowering_input_output_aliases=(),
            sim_require_finite=True, sim_require_nnan=True, nc=nc))

    devices = jax.devices()[:N_CORES]
    mesh = Mesh(np.asarray(devices), ("core",))
    n_in = len(in_names) + len(zero_outs)
    sharded = jax.jit(
        shard_map(_body, mesh=mesh, in_specs=(PartitionSpec("core"),) * n_in,
                  out_specs=(PartitionSpec("core"),) * len(out_names),
                  check_rep=False),
        keep_unused=True)
    sh = NamedSharding(mesh, PartitionSpec("core"))
    concat_in = [np.concatenate([np.asarray(in_maps[c][nm])
                                 for c in range(N_CORES)], axis=0)
                 for nm in in_names]
    dev_in = ([jax.device_put(x, sh) for x in concat_in]
              + [jax.device_put(z, sh) for z in zero_outs])
    jax.block_until_ready(dev_in)
    state = {}

    def run_n(n):
        outs = [sharded(*dev_in) for _ in range(n)]
        jax.block_until_ready(outs)
        state["last"] = outs[-1]

    def fetch():
        oi = out_names.index("out")
        full = np.asarray(state["last"][oi]).reshape(N_CORES, N_AGENTS, SH)
        q = np.concatenate([full[c] for c in range(N_CORES)], axis=1)
        return q[..., None].astype(np.float32)

    return run_n, fetch


if __name__ == "__main__":
    import reference as R
    inp = {k: np.asarray(v) for k, v in R.setup_inputs().items()}
    got = kernel(**inp)
    print("kernel out", got.shape)



# revision 3
# speedup vs baseline: 798.1701x; 1.3034x over previous
"""Trainium2 Bass kernel for nn_Attention_Critic (8-agent attention critic).

Data-parallel over batch across 8 NeuronCores (2048 batch rows per core).
BatchNorm statistics are made global via a tiny AllReduce (8x160 f32).

Device layout: feature-on-partitions, [feat, batch] tiles.
  - host pre-transposes s|a to [agent, 80, 2048] per core (bf16)
  - encoders / projections: PE matmuls, contraction on partitions
  - attention einsum pibd,pjbd->pijb: DVE pairwise muls on [128=(head,d), B]
    tiles + PE block-ones matmuls reducing over d (partition reduce) into a
    compact [32=(4j+p), B] logits block
  - softmax: ACT exp, PE ones-matmul for the j-sum, DVE reciprocal; weights
    broadcast d-wise back to [128, B] by a second ones-pattern matmul
  - weighted sum over agents folded into the critic's PSUM accumulation
  - argmax-gather of q: exact first-max logic with DVE compare/reduce ops
"""

import sys

sys.path.insert(0, "/opt/trn_rl_repo")

import numpy as np
from ml_dtypes import bfloat16

from concourse import bacc, bass, mybir, tile
from concourse.bass_utils import run_bass_kernel_spmd

F32 = mybir.dt.float32
BF16 = mybir.dt.bfloat16
AF = mybir.ActivationFunctionType
ALU = mybir.AluOpType
AX = mybir.AxisListType

N_AGENTS = 8
BATCH = 16384
SDIM, ADIM = 64, 16
CDIM = SDIM + ADIM  # 80
HID = 128
HEADS = 4
ATT_D = 32
N_CORES = 8
SH = BATCH // N_CORES  # 2048 batch rows per core
NH = 2  # batch halves per core
BH = SH // NH  # 1024
NC512 = BH // 512  # matmul chunks per half
EPS = 1e-5
ISQD = float(1.0 / np.sqrt(np.float32(ATT_D)))
SLOPE = 0.01  # LeakyReLU negative slope

_CACHE = {}
DBG = False


def _build():
    nc = bacc.Bacc(None, num_devices=N_CORES)

    # ---- DRAM parameters (per-core shard shapes) ----
    xt = nc.declare_dram_parameter("xt", [N_AGENTS, CDIM, SH], BF16, isOutput=False)
    an = nc.declare_dram_parameter("an", [N_AGENTS, SH, ADIM], F32, isOutput=False)
    wsa = nc.declare_dram_parameter("wsa", [N_AGENTS, CDIM, HID], BF16, isOutput=False)
    wse = nc.declare_dram_parameter("wse", [N_AGENTS, SDIM, HID], BF16, isOutput=False)
    wk = nc.declare_dram_parameter("wk", [HID, HID], BF16, isOutput=False)
    wsl = nc.declare_dram_parameter("wsl", [HID, HID], BF16, isOutput=False)
    wv = nc.declare_dram_parameter("wv", [HID, HID], BF16, isOutput=False)
    wc1a = nc.declare_dram_parameter("wc1a", [N_AGENTS, HID, HID], BF16, isOutput=False)
    wc1b = nc.declare_dram_parameter("wc1b", [N_AGENTS, HID, HID], BF16, isOutput=False)
    wc2 = nc.declare_dram_parameter("wc2", [N_AGENTS, HID, ADIM], BF16, isOutput=False)
    bsa = nc.declare_dram_parameter("bsa", [N_AGENTS, HID], F32, isOutput=False)
    bse = nc.declare_dram_parameter("bse", [N_AGENTS, HID], F32, isOutput=False)
    bvv = nc.declare_dram_parameter("bvv", [HID], F32, isOutput=False)
    bc1 = nc.declare_dram_parameter("bc1", [N_AGENTS, HID], F32, isOutput=False)
    bc2 = nc.declare_dram_parameter("bc2", [N_AGENTS, ADIM], F32, isOutput=False)
    cred = nc.declare_dram_parameter("cred", [HID, 32], BF16, isOutput=False)
    cz = nc.declare_dram_parameter("cz", [HID, HEADS], BF16, isOutput=False)
    crep = nc.declare_dram_parameter("crep", [HEADS, HID], F32, isOutput=False)
    cbc = nc.declare_dram_parameter("cbc", [4, HID, HID], BF16, isOutput=False)
    crow = nc.declare_dram_parameter("crow", [1, HID], BF16, isOutput=False)
    crev = nc.declare_dram_parameter("crev", [128, ADIM], F32, isOutput=False)
    out = nc.declare_dram_parameter("out", [N_AGENTS, SH], F32, isOutput=True)
    dbg = {}
    if DBG:
        for nm, shape in [("stats", [CDIM, 16]), ("san", [CDIM, BH]),
                          ("e", [HID, BH]), ("se", [HID, BH]),
                          ("keys", [HID, BH]), ("sel", [HID, BH]),
                          ("vals", [HID, BH]), ("ex", [HID, BH]),
                          ("wg", [HID, BH]), ("hh", [HID, BH]),
                          ("aq", [128, 8 * ADIM]), ("rzz", [HEADS, BH])]:
            dbg[nm] = nc.declare_dram_parameter("dbg_" + nm, shape, F32,
                                                isOutput=True)

    # internal DRAM for the BN-stats AllReduce (sum | sumsq per agent)
    stats_in = nc.dram_tensor("stats_in", [CDIM, 2 * N_AGENTS], F32)
    stats_out = nc.dram_tensor(
        "stats_out", [CDIM, 2 * N_AGENTS], F32, addr_space="Shared"
    )

    with tile.TileContext(nc) as tc:
        with (
            tc.tile_pool(name="wpool", bufs=1) as wp,     # resident weights/consts
            tc.tile_pool(name="xpool", bufs=1) as xp,     # xt tiles (phase 1+2)
            tc.tile_pool(name="stat", bufs=1) as stp,    # small per-agent vectors
            tc.tile_pool(name="work", bufs=3) as wkp,     # big transient tiles
            tc.tile_pool(name="kvpool", bufs=1) as kvp,   # keys/sel/vals/se per half
            tc.tile_pool(name="attn", bufs=2) as atp,     # exp/w tiles
            tc.tile_pool(name="qp", bufs=4) as qp,        # q path tiles
            tc.tile_pool(name="ps", bufs=3, space="PSUM") as ps,    # [128,1024] slots
            tc.tile_pool(name="wbps", bufs=2, space="PSUM") as wbp,  # [128,512] slots
        ):
            # ---------- resident weights ----------
            w_sa = [wp.tile([CDIM, HID], BF16, tag=f"wsa{n}", name=f"wsa{n}") for n in range(N_AGENTS)]
            w_se = [wp.tile([SDIM, HID], BF16, tag=f"wse{n}", name=f"wse{n}") for n in range(N_AGENTS)]
            w_k = wp.tile([HID, HID], BF16, tag="wk", name="wk")
            w_sl = wp.tile([HID, HID], BF16, tag="wsl", name="wsl")
            w_v = wp.tile([HID, HID], BF16, tag="wv", name="wv")
            w_c1a = [wp.tile([HID, HID], BF16, tag=f"wc1a{n}", name=f"wc1a{n}") for n in range(N_AGENTS)]
            w_c1b = [wp.tile([HID, HID], BF16, tag=f"wc1b{n}", name=f"wc1b{n}") for n in range(N_AGENTS)]
            w_c2 = [wp.tile([HID, ADIM], BF16, tag=f"wc2{n}", name=f"wc2{n}") for n in range(N_AGENTS)]
            b_sa = [wp.tile([HID, 1], F32, tag=f"bsa{n}", name=f"bsa{n}") for n in range(N_AGENTS)]
            b_se = [wp.tile([HID, 1], F32, tag=f"bse{n}", name=f"bse{n}") for n in range(N_AGENTS)]
            b_v = wp.tile([HID, 1], F32, tag="bv", name="bv")
            b_c1 = [wp.tile([HID, 1], F32, tag=f"bc1{n}", name=f"bc1{n}") for n in range(N_AGENTS)]
            b_c2f = [wp.tile([1, ADIM], F32, tag=f"bc2f{n}", name=f"bc2f{n}") for n in range(N_AGENTS)]
            b_c2r = [wp.tile([1, ADIM], BF16, tag=f"bc2{n}", name=f"bc2{n}") for n in range(N_AGENTS)]
            for n in range(N_AGENTS):
                nc.sync.dma_start(w_sa[n][:], wsa[n])
                nc.sync.dma_start(w_se[n][:], wse[n])
                nc.sync.dma_start(w_c1a[n][:], wc1a[n])
                nc.sync.dma_start(w_c1b[n][:], wc1b[n])
                nc.sync.dma_start(w_c2[n][:], wc2[n])
                nc.sync.dma_start(b_sa[n][:], bsa[n].unsqueeze(1))
                nc.sync.dma_start(b_se[n][:], bse[n].unsqueeze(1))
                nc.sync.dma_start(b_c1[n][:], bc1[n].unsqueeze(1))
                nc.sync.dma_start(b_c2f[n][:], bc2[n].unsqueeze(0))
                nc.vector.tensor_copy(b_c2r[n][:], b_c2f[n][:])
            nc.sync.dma_start(w_k[:], wk[:])
            nc.sync.dma_start(w_sl[:], wsl[:])
            nc.sync.dma_start(w_v[:], wv[:])
            nc.sync.dma_start(b_v[:], bvv[:].unsqueeze(1))

            # ---------- constant stationary matrices (host-built) ----------
            ones_red = wp.tile([HID, 32], BF16, tag="onesred", name="onesred")
            nc.sync.dma_start(ones_red[:], cred[:])
            ones_z = wp.tile([HID, HEADS], BF16, tag="onesz", name="onesz")
            nc.sync.dma_start(ones_z[:], cz[:])
            ones_rep = wp.tile([HEADS, HID], F32, tag="onesrep", name="onesrep")
            nc.sync.dma_start(ones_rep[:], crep[:])
            st_bc = []
            for j in range(4):
                t = wp.tile([HID, HID], BF16, tag=f"stbc{j}", name=f"stbc{j}")
                nc.sync.dma_start(t[:], cbc[j])
                st_bc.append(t)
            ones_row = wp.tile([1, HID], BF16, tag="onesrow", name="onesrow")
            nc.sync.dma_start(ones_row[:], crow[:])
            rev_f = wp.tile([128, ADIM], F32, tag="revf", name="revf")
            nc.sync.dma_start(rev_f[:], crev[:])

            # bc2 broadcast down partitions: [128, 16] = ones_row.T @ bc2_row
            bc2b = [wp.tile([128, ADIM], F32, tag=f"bc2b{n}", name=f"bc2b{n}") for n in range(N_AGENTS)]
            for n in range(N_AGENTS):
                bc2_ps = wbp.tile([128, ADIM], F32, tag="wb", name="bc2ps")
                nc.tensor.matmul(bc2_ps[:], ones_row[:], b_c2r[n][:],
                                 start=True, stop=True)
                nc.vector.tensor_copy(bc2b[n][:], bc2_ps[:])

            # ---------- phase 1: BN statistics + AllReduce ----------
            xts = []
            stats_sb = stp.tile([CDIM, 2 * N_AGENTS], F32, tag="statssb",
                                name="statssb")
            for n in range(N_AGENTS):
                xt_n = xp.tile([CDIM, SH], BF16, tag=f"xt{n}", name=f"xt{n}")
                nc.sync.dma_start(xt_n[:], xt[n])
                xts.append(xt_n)
                nc.vector.tensor_reduce(stats_sb[:, 2 * n:2 * n + 1], xt_n[:],
                                        AX.X, ALU.add)
                scr2 = wkp.tile([CDIM, SH], BF16, tag="scr", name="scr")
                nc.scalar.activation(scr2[:], xt_n[:], AF.Square,
                                     accum_out=stats_sb[:, 2 * n + 1:2 * n + 2])
            nc.sync.dma_start(stats_in[:], stats_sb[:])

            nc.gpsimd.collective_compute(
                "AllReduce", ALU.add,
                replica_groups=[list(range(N_CORES))],
                ins=[stats_in[:]],
                outs=[stats_out[:]],
            )

            # global mean/rstd per agent
            rstds, nbs = [], []
            w_sa2 = [wp.tile([CDIM, HID], BF16, tag=f"wsa2{n}", name=f"wsa2{n}")
                     for n in range(N_AGENTS)]
            w_se2 = [wp.tile([SDIM, HID], BF16, tag=f"wse2{n}", name=f"wse2{n}")
                     for n in range(N_AGENTS)]
            b_sa2 = [wp.tile([HID, 1], F32, tag=f"bsa2{n}", name=f"bsa2{n}")
                     for n in range(N_AGENTS)]
            b_se2 = [wp.tile([HID, 1], F32, tag=f"bse2{n}", name=f"bse2{n}")
                     for n in range(N_AGENTS)]
            gs_all = stp.tile([CDIM, 2 * N_AGENTS], F32, tag="gsall", name="gsall")
            nc.sync.dma_start(gs_all[:], stats_out[:])
            if DBG:
                nc.sync.dma_start(dbg["stats"][:], gs_all[:])
            for n in range(N_AGENTS):
                mean = stp.tile([CDIM, 1], F32, tag=f"mean{n}", name=f"mean{n}")
                nc.vector.tensor_scalar_mul(mean[:], gs_all[:, 2 * n:2 * n + 1],
                                            1.0 / BATCH)
                ex2 = stp.tile([CDIM, 1], F32, tag=f"ex2{n}", name=f"ex2{n}")
                nc.vector.tensor_scalar_mul(ex2[:], gs_all[:, 2 * n + 1:2 * n + 2],
                                            1.0 / BATCH)
                var = stp.tile([CDIM, 1], F32, tag=f"var{n}", name=f"var{n}")
                nc.vector.tensor_tensor(var[:], mean[:], mean[:], ALU.mult)
                nc.vector.tensor_tensor(var[:], ex2[:], var[:], ALU.subtract)
                nc.vector.tensor_scalar_add(var[:], var[:], EPS)
                ivar = stp.tile([CDIM, 1], F32, tag=f"ivar{n}", name=f"ivar{n}")
                nc.vector.reciprocal(ivar[:], var[:])
                rstd = stp.tile([CDIM, 1], F32, tag=f"rstd{n}", name=f"rstd{n}")
                nc.scalar.activation(rstd[:], ivar[:], AF.Sqrt)
                nb = stp.tile([CDIM, 1], F32, tag=f"nb{n}", name=f"nb{n}")
                nc.vector.tensor_tensor(nb[:], mean[:], rstd[:], ALU.mult)
                nc.vector.tensor_scalar_mul(nb[:], nb[:], -1.0)
                rstds.append(rstd)
                nbs.append(nb)
                # fold BN affine into the encoder weights:
                # e = lrelu(W^T((x-m)*r) + b) = lrelu((diag(r)W)^T x + (b - W^T(m*r)))
                mr = stp.tile([CDIM, 1], F32, tag=f"mr{n}", name=f"mr{n}")
                nc.vector.tensor_tensor(mr[:], mean[:], rstd[:], ALU.mult)
                mrb = stp.tile([CDIM, 1], BF16, tag=f"mrb{n}", name=f"mrb{n}")
                nc.vector.tensor_copy(mrb[:], mr[:])
                nc.vector.tensor_scalar_mul(w_sa2[n][:], w_sa[n][:], rstd[:])
                nc.vector.tensor_scalar_mul(w_se2[n][:], w_se[n][:], rstd[0:SDIM, :])
                bo_ps = wbp.tile([HID, 1], F32, tag="wb", name="bo_ps")
                nc.tensor.matmul(bo_ps[:], w_sa[n][:], mrb[:], start=True, stop=True)
                nc.vector.tensor_tensor(b_sa2[n][:], b_sa[n][:], bo_ps[:],
                                        ALU.subtract)
                bo_ps2 = wbp.tile([HID, 1], F32, tag="wb", name="bo_ps2")
                nc.tensor.matmul(bo_ps2[:], w_se[n][:], mrb[0:SDIM, :],
                                 start=True, stop=True)
                nc.vector.tensor_tensor(b_se2[n][:], b_se[n][:], bo_ps2[:],
                                        ALU.subtract)

            # ---------- phases 2-4 per batch-half ----------
            for h in range(NH):
                hs = h * BH
                # phase 2: encoders -> keys/sel/vals/se for all agents
                keys, sel, vals, se = [], [], [], []
                for n in range(N_AGENTS):
                    xv = xts[n][:, hs:hs + BH]
                    e_ps = ps.tile([HID, BH], F32, tag="ps", name="ps")
                    for c in range(NC512):
                        cs = slice(512 * c, 512 * (c + 1))
                        nc.tensor.matmul(e_ps[:, cs], w_sa2[n][:], xv[:, cs],
                                         start=True, stop=True)
                    e_n = wkp.tile([HID, BH], BF16, tag="en", name="en")
                    nc.scalar.activation(e_n[:], e_ps[:], AF.Lrelu, bias=b_sa2[n][:],
                                         alpha=SLOPE)
                    se_ps = ps.tile([HID, BH], F32, tag="ps", name="ps")
                    for c in range(NC512):
                        cs = slice(512 * c, 512 * (c + 1))
                        nc.tensor.matmul(se_ps[:, cs], w_se2[n][:],
                                         xv[0:SDIM, cs], start=True, stop=True)
                    se_n = kvp.tile([HID, BH], BF16, tag=f"se{n}", name=f"se{n}")
                    nc.scalar.activation(se_n[:], se_ps[:], AF.Lrelu, bias=b_se2[n][:],
                                         alpha=SLOPE)
                    se.append(se_n)
                    k_ps = ps.tile([HID, BH], F32, tag="ps", name="ps")
                    for c in range(NC512):
                        cs = slice(512 * c, 512 * (c + 1))
                        nc.tensor.matmul(k_ps[:, cs], w_k[:], e_n[:, cs],
                                         start=True, stop=True)
                    k_n = kvp.tile([HID, BH], BF16, tag=f"k{n}", name=f"k{n}")
                    nc.scalar.copy(k_n[:], k_ps[:])
                    keys.append(k_n)
                    sl_ps = ps.tile([HID, BH], F32, tag="ps", name="ps")
                    for c in range(NC512):
                        cs = slice(512 * c, 512 * (c + 1))
                        nc.tensor.matmul(sl_ps[:, cs], w_sl[:], se_n[:, cs],
                                         start=True, stop=True)
                    sl_n = kvp.tile([HID, BH], BF16, tag=f"sl{n}", name=f"sl{n}")
                    nc.scalar.copy(sl_n[:], sl_ps[:])
                    sel.append(sl_n)
                    v_ps = ps.tile([HID, BH], F32, tag="ps", name="ps")
                    for c in range(NC512):
                        cs = slice(512 * c, 512 * (c + 1))
                        nc.tensor.matmul(v_ps[:, cs], w_v[:], e_n[:, cs],
                                         start=True, stop=True)
                    v_n = kvp.tile([HID, BH], BF16, tag=f"v{n}", name=f"v{n}")
                    nc.scalar.activation(v_n[:], v_ps[:], AF.Lrelu, bias=b_v[:],
                                         alpha=SLOPE)
                    vals.append(v_n)
                    if DBG and h == 0 and n == 0:
                        stg = wkp.tile([HID, BH], F32, tag="dbgstg", name="dbgstg")
                        for nm, tl in [("san", sa_n), ("e", e_n), ("se", se_n),
                                       ("keys", k_n), ("sel", sl_n), ("vals", v_n)]:
                            if nm == "san":
                                nc.vector.tensor_copy(stg[0:CDIM, :], tl[:])
                                nc.sync.dma_start(dbg[nm][:], stg[0:CDIM, :])
                            else:
                                nc.vector.tensor_copy(stg[:], tl[:])
                                nc.sync.dma_start(dbg[nm][:], stg[:])
                            stg = wkp.tile([HID, BH], F32, tag="dbgstg",
                                           name="dbgstg")

                # phases 3+4: attention + critic + q, per agent i
                for i in range(N_AGENTS):
                    jall = [j for j in range(N_AGENTS) if j != i]
                    # --- logits: two [128,BH] psum tiles (j 0-3 | j 4-7), row
                    # block 32*(j%4) holds pair (i,j); diag computed then zeroed
                    lgA = ps.tile([HID, BH], F32, tag="ps", name="lgA")
                    lgB = ps.tile([HID, BH], F32, tag="ps", name="lgB")
                    for j in range(N_AGENTS):
                        if j == i:
                            continue
                        prod = wkp.tile([HID, BH], BF16, tag="prod", name="prod")
                        nc.vector.tensor_tensor(prod[:], sel[i][:], keys[j][:],
                                                ALU.mult)
                        lg = lgA if j < 4 else lgB
                        jj = j % 4
                        for c in range(NC512):
                            cs = slice(512 * c, 512 * (c + 1))
                            nc.tensor.matmul(lg[32 * jj:32 * (jj + 1), cs],
                                             ones_red[:], prod[:, cs],
                                             start=True, stop=True,
                                             tile_position=(0, 32 * jj))
                    # --- exp (scaled); diagonal row-block zeroed after ---
                    exA = atp.tile([HID, BH], BF16, tag="exA", name="exA")
                    exB = atp.tile([HID, BH], BF16, tag="exB", name="exB")
                    nc.scalar.activation(exA[:], lgA[:], AF.Exp, scale=ISQD)
                    nc.scalar.activation(exB[:], lgB[:], AF.Exp, scale=ISQD)
                    exd = exA if i < 4 else exB
                    nc.vector.memset(exd[32 * (i % 4):32 * (i % 4 + 1), :], 0.0)
                    # --- Z = sum_j exp -> [4, BH]; w = exp / Z ---
                    z_ps = ps.tile([HEADS, BH], F32, tag="ps", name="zps")
                    for c in range(NC512):
                        cs = slice(512 * c, 512 * (c + 1))
                        nc.tensor.matmul(z_ps[:, cs], ones_z[:], exA[:, cs],
                                         start=True, stop=False)
                        nc.tensor.matmul(z_ps[:, cs], ones_z[:], exB[:, cs],
                                         start=False, stop=True)
                    rz = atp.tile([HEADS, BH], F32, tag="rz", name="rz")
                    nc.vector.reciprocal(rz[:], z_ps[:])
                    rzr_ps = ps.tile([HID, BH], F32, tag="ps", name="rzrps")
                    for c in range(NC512):
                        cs = slice(512 * c, 512 * (c + 1))
                        nc.tensor.matmul(rzr_ps[:, cs], ones_rep[:], rz[:, cs],
                                         start=True, stop=True)
                    rzr_sb = atp.tile([HID, BH], BF16, tag="rzrsb", name="rzrsb")
                    nc.scalar.copy(rzr_sb[:], rzr_ps[:])
                    wgA = atp.tile([HID, BH], BF16, tag="wgA", name="wgA")
                    wgB = atp.tile([HID, BH], BF16, tag="wgB", name="wgB")
                    nc.vector.tensor_tensor(wgA[:], exA[:], rzr_sb[:], ALU.mult)
                    nc.vector.tensor_tensor(wgB[:], exB[:], rzr_sb[:], ALU.mult)
                    # --- critic h: Wc1a^T se_i + sum_j Wc1b^T (bcast(w_ij) * v_j) ---
                    h_ps = ps.tile([HID, BH], F32, tag="ps", name="h_ps")
                    for c in range(NC512):
                        cs = slice(512 * c, 512 * (c + 1))
                        nc.tensor.matmul(h_ps[:, cs], w_c1a[i][:], se[i][:, cs],
                                         start=True, stop=False)
                        for idx, j in enumerate(jall):
                            wsrc = wgA if j < 4 else wgB
                            wb_ps = wbp.tile([HID, 512], F32, tag="wb", name="wb")
                            nc.tensor.matmul(wb_ps[:], st_bc[j % 4][:],
                                             wsrc[:, cs], start=True, stop=True)
                            wv_t = wkp.tile([HID, 512], BF16, tag="wvt", name="wvt")
                            nc.vector.tensor_tensor(wv_t[:], vals[j][:, cs],
                                                    wb_ps[:], ALU.mult)
                            nc.tensor.matmul(h_ps[:, cs], w_c1b[i][:], wv_t[:],
                                             start=False, stop=(idx == len(jall) - 1))
                    h_i = wkp.tile([HID, BH], BF16, tag="hi", name="hi")
                    nc.scalar.activation(h_i[:], h_ps[:], AF.Lrelu, bias=b_c1[i][:],
                                         alpha=SLOPE)
                    if DBG and h == 0 and i == 0:
                        stg = wkp.tile([HID, BH], F32, tag="dbgstg", name="dbgstg")
                        for nm, tl in [("ex", exA), ("wg", wgA), ("hh", h_i)]:
                            nc.vector.tensor_copy(stg[:], tl[:])
                            nc.sync.dma_start(dbg[nm][:], stg[:])
                            stg = wkp.tile([HID, BH], F32, tag="dbgstg",
                                           name="dbgstg")
                        nc.vector.tensor_copy(stg[0:HEADS, :], rz[:])
                        nc.sync.dma_start(dbg["rzz"][:], stg[0:HEADS, :])
                    # --- all_q natural layout via stationary-activation matmul ---
                    aq_ps = wbp.tile([128, 8 * ADIM], F32, tag="wb", name="aq")
                    for t in range(8):  # 8 b-tiles of 128 in this half
                        nc.tensor.matmul(aq_ps[:, ADIM * t:ADIM * (t + 1)],
                                         h_i[:, 128 * t:128 * (t + 1)], w_c2[i][:],
                                         start=True, stop=True)
                    aq = qp.tile([128, 8 * ADIM], F32, tag="aqsb", name="aqsb")
                    aq3 = aq[:].rearrange("p (t k) -> p t k", t=8)
                    nc.vector.tensor_tensor(
                        aq3, aq_ps[:].rearrange("p (t k) -> p t k", t=8),
                        bc2b[i][:].unsqueeze(1).broadcast_to([128, 8, ADIM]),
                        ALU.add)
                    if DBG and h == 0 and i == 0:
                        nc.sync.dma_start(dbg["aq"][:], aq[:])
                    # --- exact argmax(a) one-hot and gather ---
                    a8 = qp.tile([128, 8 * ADIM], F32, tag="a8", name="a8")
                    nc.sync.dma_start(
                        a8[:].rearrange("p (t k) -> p t k", t=8),
                        an[i, hs:hs + BH, :].rearrange("(t p) k -> p t k", p=128))
                    a83 = a8[:].rearrange("p (t k) -> p t k", t=8)
                    amax = qp.tile([128, 8], F32, tag="amax", name="amax")
                    nc.vector.tensor_reduce(amax[:], a83, AX.X, ALU.max)
                    eq = qp.tile([128, 8 * ADIM], F32, tag="eq", name="eq")
                    eq3 = eq[:].rearrange("p (t k) -> p t k", t=8)
                    nc.vector.tensor_tensor(
                        eq3, a83, amax[:].unsqueeze(2).broadcast_to([128, 8, ADIM]),
                        ALU.is_equal)
                    nc.vector.tensor_tensor(
                        eq3, eq3, rev_f[:].unsqueeze(1).broadcast_to([128, 8, ADIM]),
                        ALU.mult)
                    smax = qp.tile([128, 8], F32, tag="smax", name="smax")
                    nc.vector.tensor_reduce(smax[:], eq3, AX.X, ALU.max)
                    nc.vector.tensor_tensor(
                        eq3, eq3, smax[:].unsqueeze(2).broadcast_to([128, 8, ADIM]),
                        ALU.is_equal)
                    nc.vector.tensor_tensor(eq3, eq3, aq3, ALU.mult)
                    q_i = qp.tile([128, 8], F32, tag="qi", name="qi")
                    nc.vector.tensor_reduce(q_i[:], eq3, AX.X, ALU.add)
                    nc.sync.dma_start(
                        out[i, hs:hs + BH].rearrange("(t p) -> p t", p=128),
                        q_i[:])

    nc.compile()
    return nc


def _get_nc():
    if "nc" not in _CACHE:
        _CACHE["nc"] = _build()
    return _CACHE["nc"]


def make_in_maps(s, a, W_sa, b_sa, W_se, b_se, Wk, Wsel, Wv, bv, Wc1, bc1, Wc2, bc2):
    s = np.asarray(s, np.float32)
    a = np.asarray(a, np.float32)
    x = np.concatenate([s, a], axis=-1)  # [8, 16384, 80]

    def b16(v):
        return np.ascontiguousarray(np.asarray(v, np.float32).astype(bfloat16))

    ones_red = np.zeros((HID, 32), np.float32)
    ones_z = np.zeros((HID, HEADS), np.float32)
    ones_rep = np.zeros((HEADS, HID), np.float32)
    st_bc = np.zeros((4, HID, HID), np.float32)
    for p in range(HEADS):
        ones_red[32 * p:32 * (p + 1), 8 * p:8 * (p + 1)] = 1.0
        for j in range(4):
            ones_z[32 * j + 8 * p, p] = 1.0
            ones_rep[p, 32 * j + 8 * p:32 * j + 8 * p + 8] = 1.0
            st_bc[j, 32 * j + 8 * p, 32 * p:32 * (p + 1)] = 1.0
    rev = np.tile(np.arange(ADIM, 0, -1, dtype=np.float32), (128, 1))
    shared = {
        "cred": b16(ones_red), "cz": b16(ones_z),
        "crep": np.ascontiguousarray(ones_rep),
        "cbc": b16(st_bc), "crow": b16(np.ones((1, HID), np.float32)),
        "crev": np.ascontiguousarray(rev),
        "wsa": b16(W_sa), "wse": b16(W_se),
        "wk": b16(np.transpose(np.asarray(Wk, np.float32), (1, 0, 2)).reshape(HID, HID)),
        "wsl": b16(np.transpose(np.asarray(Wsel, np.float32), (1, 0, 2)).reshape(HID, HID)),
        "wv": b16(np.transpose(np.asarray(Wv, np.float32), (1, 0, 2)).reshape(HID, HID)),
        "wc1a": b16(np.asarray(Wc1, np.float32)[:, :HID, :]),
        "wc1b": b16(np.asarray(Wc1, np.float32)[:, HID:, :]),
        "wc2": b16(Wc2),
        "bsa": np.ascontiguousarray(np.asarray(b_sa, np.float32)),
        "bse": np.ascontiguousarray(np.asarray(b_se, np.float32)),
        "bvv": np.ascontiguousarray(np.asarray(bv, np.float32).reshape(HID)),
        "bc1": np.ascontiguousarray(np.asarray(bc1, np.float32)),
        "bc2": np.ascontiguousarray(np.asarray(bc2, np.float32)),
    }
    in_maps = []
    for c in range(N_CORES):
        sl = slice(c * SH, (c + 1) * SH)
        m = dict(shared)
        m["xt"] = np.ascontiguousarray(
            np.transpose(x[:, sl, :], (0, 2, 1)).astype(np.float32)).astype(bfloat16)
        m["an"] = np.ascontiguousarray(a[:, sl, :])
        in_maps.append(m)
    return in_maps


def kernel(**inputs):
    in_maps = make_in_maps(**inputs)
    nc = _get_nc()
    res = run_bass_kernel_spmd(nc, in_maps, core_ids=list(range(N_CORES)))
    outs = [np.asarray(res.results[c]["out"]) for c in range(N_CORES)]
    q = np.concatenate(outs, axis=1)  # [8, 16384]
    return q[..., None].astype(np.float32)


def make_runner(in_maps):
    """Build the sharded PJRT executable ONCE and pre-stage the per-core
    inputs on the 8 devices. Returns (run_n, fetch) where run_n(n)
    dispatches n back-to-back executions of the NEFF and blocks until all
    complete, and fetch() returns the [8,16384,1] output of the last run.

    This is the measurement path: executable reuse + device-resident
    inputs isolate NEFF execution from per-call jit re-tracing, BIR
    recompilation and host->device staging that run_bass_kernel_spmd
    re-pays on every invocation.
    """
    import jax
    from jax.sharding import Mesh, NamedSharding, PartitionSpec
    from jax.experimental.shard_map import shard_map
    from concourse.bass2jax import (
        _bass_exec_p, partition_id_tensor, install_neuronx_cc_hook)

    install_neuronx_cc_hook()
    nc = _get_nc()
    partition_name = (nc.partition_id_tensor.name
                      if nc.partition_id_tensor else None)
    in_names, out_names, out_avals, zero_outs = [], [], [], []
    for alloc in nc.m.functions[0].allocations:
        if not isinstance(alloc, mybir.MemoryLocationSet):
            continue
        name = alloc.memorylocations[0].name
        if alloc.kind == "ExternalInput":
            if name != partition_name:
                in_names.append(name)
        elif alloc.kind == "ExternalOutput":
            shape = tuple(alloc.tensor_shape)
            dtype = mybir.dt.np(alloc.dtype)
            out_names.append(name)
            out_avals.append(jax.core.ShapedArray(shape, dtype))
            zero_outs.append(np.zeros((N_CORES * shape[0], *shape[1:]), dtype))
    in_names_all = in_names + out_names + (
        [partition_name] if partition_name else [])

    def _body(*args):
        operands = list(args)
        if partition_name is not None:
            operands.append(partition_id_tensor())
        return tuple(_bass_exec_p.bind(
            *operands, out_avals=tuple(out_avals), in_names=tuple(in_names_all),
            out_names=tuple(out_names), lowering_input_output_aliases=(),
            sim_require_finite=True, sim_require_nnan=True, nc=nc))

    devices = jax.devices()[:N_CORES]
    mesh = Mesh(np.asarray(devices), ("core",))
    n_in = len(in_names) + len(zero_outs)
    sharded = jax.jit(
        shard_map(_body, mesh=mesh, in_specs=(PartitionSpec("core"),) * n_in,
                  out_specs=(PartitionSpec("core"),) * len(out_names),
                  check_rep=False),
        keep_unused=True)
    sh = NamedSharding(mesh, PartitionSpec("core"))
    concat_in = [np.concatenate([np.asarray(in_maps[c][nm])
                                 for c in range(N_CORES)], axis=0)
                 for nm in in_names]
    dev_in = ([jax.device_put(x, sh) for x in concat_in]
              + [jax.device_put(z, sh) for z in zero_outs])
    jax.block_until_ready(dev_in)
    state = {}

    def run_n(n):
        outs = [sharded(*dev_in) for _ in range(n)]
        jax.block_until_ready(outs)
        state["last"] = outs[-1]

    def fetch():
        oi = out_names.index("out")
        full = np.asarray(state["last"][oi]).reshape(N_CORES, N_AGENTS, SH)
        q = np.concatenate([full[c] for c in range(N_CORES)], axis=1)
        return q[..., None].astype(np.float32)

    return run_n, fetch


if __name__ == "__main__":
    import reference as R
    inp = {k: np.asarray(v) for k, v in R.setup_inputs().items()}
    got = kernel(**inp)
    print("kernel out", got.shape)

